# revision 1
# baseline (speedup 1.0000x reference)
"""Causal self-attention (B=4, T=2048, C=1024, H=16, D=64) on 8 TRN2 NeuronCores.

Sharding: batch x head-group. Core c handles batch b = c//2 and heads
hg*8..hg*8+8 where hg = c%2 (data parallel on batch, tensor parallel on heads;
w_qkv column-sharded, w_out row-sharded). Each core is fully independent; the
host sums the two per-batch partial outputs and adds the bias terms.

All matmuls run in float32r (fp32 with the mantissa RNE-rounded to 11 explicit
bits; full 1 column/cycle PE rate for moving dims >= 256). DRAM inputs that
feed matmuls are pre-rounded to the fp32r bit format on the host and declared
float32r end to end; every on-chip matmul operand is produced by an
instruction whose output dtype is float32r, which performs the rounding.

Per-core kernel:
  Phase B: QKV projection. x arrives pre-transposed (xT [C,T]) so
    Q^T/K^T [qk-col, t] come out of the PE directly (lhsT=w1, rhs=xT), and
    V [t, v-col] comes from the swapped orientation (lhsT=xT, rhs=w1v).
    V is stored packed per t-tile as [128, 8*(64+1)] with a ones column per
    head, which makes the PV matmul also emit the softmax denominator.
  Phase C: attention per head-pair. S^T [j,t_q] = K^T.T @ Q^T (row-packed
    pairs of K=64 matmuls), exp on ScalarE straight out of PSUM into P^T in
    SBUF (no max subtraction: scores are O(1) by construction), causal
    masking via affine_select on diagonal blocks + narrowed PV ranges.
    O^T[d, t_q] (+ sum row) = [V|1].T @ P^T accumulated in PSUM. Normalize
    with reciprocal + K=1 ones-matmul broadcast + tensor_mul.
  Phase D: out projection y[t, c] = O^T.T @ w2, K=128 over pair-stacked O.
"""

import numpy as np

import concourse.bass as bass
import concourse.bacc as bacc
import concourse.mybir as mybir
from concourse.tile import TileContext

# ---- problem constants (hardcoded per contract) ----
B, T, C = 4, 2048, 1024
H_GLOBAL, D = 16, 64
HL = 8                      # local heads per core
N_CORES = 8
P = 128
KT_C = C // P               # 8 contraction tiles over C
NT = T // P                 # 16 t-tiles
IB = 512                    # query block (i-chunk)
NIC = T // IB               # 4 i-chunks
G = 2                       # j-tiles per exp group
XB = 256                    # x stream chunk width (t columns)
M1 = 3 * HL * D             # 1536 local qkv cols
F32 = mybir.dt.float32
F32R = mybir.dt.float32r
SCALE = 1.0 / np.sqrt(D).astype(np.float32)


def round_f32r(a):
    """Host-side RNE rounding of fp32 to the fp32r bit format (11-bit
    explicit mantissa). Matches libwalrus fp32_to_fp32r exactly."""
    u = np.ascontiguousarray(a, dtype=np.float32).view(np.uint32).astype(np.uint64)
    low = u & 0xFFF
    keep = u >> 12
    roundup = (low > 0x800) | ((low == 0x800) & ((keep & 1) == 1))
    return ((keep + roundup.astype(np.uint64)) << 12).astype(np.uint32).view(
        np.float32)


def build_nc(repeat=1, phases="BCD"):
    from concourse import library_config
    nc = bacc.Bacc("TRN2", target_bir_lowering=False)
    EXPF = mybir.ActivationFunctionType.Exp

    xT = nc.dram_tensor("xT", [C, T], F32R, kind="ExternalInput").ap()
    w1 = nc.dram_tensor("w1", [C, M1], F32R, kind="ExternalInput").ap()
    b1 = nc.dram_tensor("b1", [M1], F32, kind="ExternalInput").ap()
    w2 = nc.dram_tensor("w2", [HL * D, C], F32R, kind="ExternalInput").ap()
    y = nc.dram_tensor("y", [T, C], F32, kind="ExternalOutput").ap()

    with TileContext(nc) as tc:
      for _rep in range(repeat):
        with tc.tile_pool(name="persist", bufs=1) as persist:
            ones_f32 = persist.tile([P, D], F32, tag="ones_f32", name="ones_f32")
            nc.vector.memset(ones_f32[:, :], 1.0)
            ones_col = persist.tile([P, D], F32R, tag="ones", name="ones_col")
            nc.vector.tensor_copy(out=ones_col[:, :], in_=ones_f32[:, :])
            b1_sb = persist.tile([P, 12], F32, tag="b1", name="b1_sb")
            nc.sync.dma_start(out=b1_sb[:, :], in_=b1.rearrange("(m p) -> p m", p=P))

            # persistent activations
            QT = [persist.tile([P, T], F32R, tag=f"QT{pr}", name=f"QT{pr}")
                  for pr in range(HL // 2)]
            KT = [persist.tile([P, T], F32R, tag=f"KT{pr}", name=f"KT{pr}")
                  for pr in range(HL // 2)]
            V = [persist.tile([P, HL * (D + 1)], F32R, tag=f"V{tt}", name=f"V{tt}")
                 for tt in range(NT)]
            OP = [persist.tile([P, T], F32R, tag=f"OP{pr}", name=f"OP{pr}")
                  for pr in range(HL // 2)]

            # ---------------- Phase B: QKV projection ----------------
            with tc.tile_pool(name="w1p", bufs=1) as w1p, \
                 tc.tile_pool(name="xs", bufs=2) as xs, \
                 tc.tile_pool(name="ppsum", bufs=4, space="PSUM") as ppsum:
                w1_sb = w1p.tile([P, KT_C, M1], F32R, name="w1_sb")
                nc.sync.dma_start(out=w1_sb[:, :, :],
                                  in_=w1.rearrange("(k p) m -> p k m", p=P))
                for tt in range(NT):
                    nc.vector.tensor_copy(
                        out=V[tt].rearrange("p (h x) -> p h x", x=D + 1)[:, :, D:D + 1],
                        in_=ones_f32.rearrange("p (h o) -> p h o", o=1)[:, 0:HL, :])

                xT_r = xT.rearrange("(k p) t -> p k t", p=P)
                for nch in range(T // XB):
                    xc = xs.tile([P, KT_C, XB], F32R, tag="xc", name="xc")
                    nc.sync.dma_start(out=xc[:, :, :],
                                      in_=xT_r[:, :, nch * XB:(nch + 1) * XB])
                    # V for the t-tiles of this chunk: lhsT = xT, rhs = w1v
                    for ttl in range(XB // P):
                        tt = nch * (XB // P) + ttl
                        vp = ppsum.tile([P, HL * D], F32, tag="pp", name="vp")
                        for k in range(KT_C):
                            nc.tensor.matmul(
                                vp[:, :],
                                lhsT=xc[:, k, ttl * P:(ttl + 1) * P],
                                rhs=w1_sb[:, k, 2 * HL * D:3 * HL * D],
                                start=(k == 0), stop=(k == KT_C - 1))
                        nc.vector.tensor_copy(
                            out=V[tt].rearrange("p (h x) -> p h x", x=D + 1)[:, :, 0:D],
                            in_=vp.rearrange("p (h x) -> p h x", x=D))
                    # Q^T / K^T: lhsT = w1 column block, rhs = xT chunk
                    for pr in range(HL // 2):
                        qp = ppsum.tile([P, XB], F32, tag="pp", name="qp")
                        for k in range(KT_C):
                            nc.tensor.matmul(
                                qp[:, :],
                                lhsT=w1_sb[:, k, pr * P:(pr + 1) * P],
                                rhs=xc[:, k, :],
                                start=(k == 0), stop=(k == KT_C - 1))
                        nc.vector.tensor_scalar_add(
                            QT[pr][:, nch * XB:(nch + 1) * XB], qp[:, :],
                            b1_sb[:, pr:pr + 1])
                        kp = ppsum.tile([P, XB], F32, tag="pp", name="kp")
                        for k in range(KT_C):
                            nc.tensor.matmul(
                                kp[:, :],
                                lhsT=w1_sb[:, k, HL * D + pr * P:HL * D + (pr + 1) * P],
                                rhs=xc[:, k, :],
                                start=(k == 0), stop=(k == KT_C - 1))
                        nc.vector.tensor_scalar_add(
                            KT[pr][:, nch * XB:(nch + 1) * XB], kp[:, :],
                            b1_sb[:, 4 + pr:4 + pr + 1])

            # ---------------- Phase C: attention ----------------
            if "C" not in phases:
                pass
            else:
             with tc.tile_pool(name="spsum", bufs=2, space="PSUM") as spool, \
                 tc.tile_pool(name="opsum", bufs=2, space="PSUM") as opool, \
                 tc.tile_pool(name="bcpsum", bufs=2, space="PSUM") as bcp, \
                 tc.tile_pool(name="ptp", bufs=3) as ptp, \
                 tc.tile_pool(name="recp", bufs=2) as recp, \
                 tc.tile_pool(name="bncp", bufs=2) as bncp:
                for pr in range(HL // 2):
                    for ic in range(NIC):
                        njt = (ic + 1) * (IB // P)
                        ops = [opool.tile([D + 1, IB], F32, tag="op", name=f"o{h2}")
                               for h2 in range(2)]
                        ngr = (njt + G - 1) // G
                        for g in range(ngr):
                            jts = list(range(g * G, min((g + 1) * G, njt)))
                            for h2 in range(2):
                                hs = h2 * D
                                sp = spool.tile([P, G * IB], F32, tag="sp", name="sp")
                                for jl, jt in enumerate(jts):
                                    # full width: garbage j>i columns are
                                    # masked / excluded downstream
                                    nc.tensor.matmul(
                                        sp[:, jl * IB:(jl + 1) * IB],
                                        lhsT=KT[pr][hs:hs + D, jt * P:(jt + 1) * P],
                                        rhs=QT[pr][hs:hs + D,
                                                   ic * IB:(ic + 1) * IB],
                                        start=True, stop=True)
                                pt = ptp.tile([P, G * IB], F32R, tag="pt", name="pt")
                                nc.scalar.activation(pt[:, 0:len(jts) * IB],
                                                     sp[:, 0:len(jts) * IB],
                                                     EXPF, scale=float(SCALE))
                                for jl, jt in enumerate(jts):
                                    rel = jt * P - ic * IB
                                    if 0 <= rel:  # diagonal block: zero j > i
                                        nc.gpsimd.affine_select(
                                            out=pt[:, jl * IB + rel:jl * IB + rel + P],
                                            in_=pt[:, jl * IB + rel:jl * IB + rel + P],
                                            pattern=[[1, P]],
                                            compare_op=mybir.AluOpType.is_ge,
                                            fill=0.0,
                                            base=0,
                                            channel_multiplier=-1)
                                h = pr * 2 + h2
                                for jl, jt in enumerate(jts):
                                    rel = max(0, jt * P - ic * IB)
                                    nc.tensor.matmul(
                                        ops[h2][:, rel:IB],
                                        lhsT=V[jt][:, h * (D + 1):(h + 1) * (D + 1)],
                                        rhs=pt[:, jl * IB + rel:(jl + 1) * IB],
                                        start=(jt == 0), stop=(jt == njt - 1))
                        # normalize: O^T[d, i] / sum_row. Evacuate raw O^T +
                        # sums to SBUF, reciprocal the sums row, round it to
                        # f32r, broadcast across partitions with a K=1
                        # ones-matmul, multiply (rounds to f32r on write).
                        for h2 in range(2):
                            orw = recp.tile([D + 1, IB], F32, tag="orw", name="orw")
                            nc.any.tensor_copy(out=orw[:, :], in_=ops[h2][:, :])
                            nc.vector.reciprocal(orw[D:D + 1, :], orw[D:D + 1, :])
                            rcr = recp.tile([D + 1, IB], F32R, tag="rcr", name="rcr")
                            nc.any.tensor_copy(out=rcr[D:D + 1, :],
                                               in_=orw[D:D + 1, :])
                            bc = bcp.tile([D, IB], F32, tag="bc", name="bc")
                            nc.tensor.matmul(bc[:, :],
                                             lhsT=ones_col[D:D + 1, 0:D],
                                             rhs=rcr[D:D + 1, :],
                                             start=True, stop=True)
                            if h2 == 0:
                                nc.vector.tensor_mul(
                                    out=OP[pr][0:D, ic * IB:(ic + 1) * IB],
                                    in0=orw[0:D, :], in1=bc[:, :])
                            else:
                                bn = bncp.tile([D, IB], F32R, tag="bn", name="bn")
                                nc.vector.tensor_mul(out=bn[:, :],
                                                     in0=orw[0:D, :], in1=bc[:, :])
                                nc.sync.dma_start(
                                    out=OP[pr][D:2 * D, ic * IB:(ic + 1) * IB],
                                    in_=bn[:, :])

            # ---------------- Phase D: output projection ----------------
            if "D" not in phases:
                pass
            else:
             with tc.tile_pool(name="w2p", bufs=1) as w2p, \
                 tc.tile_pool(name="yout", bufs=3) as yout, \
                 tc.tile_pool(name="ypsum", bufs=4, space="PSUM") as ypsum:
                w2_sb = w2p.tile([P, HL * D // P, C], F32R, name="w2_sb")
                nc.sync.dma_start(out=w2_sb[:, :, :],
                                  in_=w2.rearrange("(k p) c -> p k c", p=P))
                for tt in range(NT):
                    for cc in range(C // 512):
                        yp = ypsum.tile([P, 512], F32, tag="yp", name="yp")
                        for pr in range(HL // 2):
                            nc.tensor.matmul(
                                yp[:, :],
                                lhsT=OP[pr][:, tt * P:(tt + 1) * P],
                                rhs=w2_sb[:, pr, cc * 512:(cc + 1) * 512],
                                start=(pr == 0), stop=(pr == HL // 2 - 1))
                        ysb = yout.tile([P, 512], F32, tag="ysb", name="ysb")
                        nc.vector.tensor_copy(out=ysb[:, :], in_=yp[:, :])
                        nc.sync.dma_start(
                            out=y[tt * P:(tt + 1) * P, cc * 512:(cc + 1) * 512],
                            in_=ysb[:, :])
    nc.compile()
    return nc


_NC_CACHE = None


def _get_nc():
    global _NC_CACHE
    if _NC_CACHE is None:
        _NC_CACHE = build_nc()
    return _NC_CACHE


def shard_inputs(x, w_qkv, b_qkv, w_out):
    """Build the 8 per-core input maps (matmul inputs pre-rounded to f32r)."""
    x = np.asarray(x, dtype=np.float32)
    w_qkv = round_f32r(np.asarray(w_qkv, dtype=np.float32))
    b_qkv = np.asarray(b_qkv, dtype=np.float32)
    w_out = round_f32r(np.asarray(w_out, dtype=np.float32))
    in_maps = []
    for core in range(N_CORES):
        b, hg = core // 2, core % 2
        cs = hg * HL * D              # 512-wide contiguous head-group slice
        w1 = np.ascontiguousarray(np.concatenate(
            [w_qkv[:, cs:cs + HL * D],
             w_qkv[:, C + cs:C + cs + HL * D],
             w_qkv[:, 2 * C + cs:2 * C + cs + HL * D]], axis=1))
        b1 = np.ascontiguousarray(np.concatenate(
            [b_qkv[cs:cs + HL * D],
             b_qkv[C + cs:C + cs + HL * D],
             b_qkv[2 * C + cs:2 * C + cs + HL * D]]))
        in_maps.append({
            "xT": round_f32r(x[b].T),
            "w1": w1,
            "b1": b1,
            "w2": np.ascontiguousarray(w_out[cs:cs + HL * D, :]),
        })
    return in_maps


def combine_outputs(results, b_qkv, w_out, b_out):
    """Sum per-batch partials from the two head-group cores + bias terms."""
    bias_vec = (np.asarray(b_qkv[2 * C:3 * C], dtype=np.float32) @
                np.asarray(w_out, dtype=np.float32) +
                np.asarray(b_out, dtype=np.float32))
    y = np.empty((B, T, C), dtype=np.float32)
    for b in range(B):
        y[b] = results[2 * b]["y"] + results[2 * b + 1]["y"] + bias_vec
    return y


def kernel(x, w_qkv, b_qkv, w_out, b_out, *, trace=False, _sink=None):
    from concourse.bass_utils import run_bass_kernel_spmd
    nc = _get_nc()
    in_maps = shard_inputs(x, w_qkv, b_qkv, w_out)
    res = run_bass_kernel_spmd(nc, in_maps, core_ids=list(range(N_CORES)),
                               trace=trace)
    if _sink is not None:
        _sink.append(res)
    return combine_outputs(res.results, b_qkv, w_out, b_out)

